# revision 24
# baseline (speedup 1.0000x reference)
"""Trainium2 Bass kernel for nn_InvDirectImageAlign (inverse-compositional image alignment).

Per core (8 cores): 2 batch elements. 5 launches of ONE compiled NEFF (one per
Gauss-Newton iteration); host does only the tiny O(B) 6x6 solve + se3_exp
between launches. Device does warp, bilinear grid_sample (GPSIMD ap_gather
from fp16 pair-dup band tables), the augmented-Jacobian build, and the
JtWJ/Rhs reduction as PE matmuls accumulated in PSUM across supersteps.

Chunking: (batch, 16-row y-band, 224-col x-half) = 80 chunks/core; the 8
GPSIMD partition-groups each own one chunk per superstep; 10 supersteps.
Gather tables are contiguous slabs of host-prepadded pair-dup half-planes
(pd: [2,12,2,H,2*(PW-1)] fp16), so each per-chunk table DMA is 12 fat
descriptors instead of ~800 row-sized ones.

JtWJ/Rhs: per pixel the 4 residual rows (3 image + 1 depth) give
J_r = a_r*A6 + b_r*B6 (+T6 for depth). With sqrt-Huber weights folded in,
jaug = [sqrt(w)*J | sqrt(w)*r] in fp16; JtWJ|Rhs = jaug^T jaug via PE
matmuls (4 a-blocks x 28 cols per matmul) accumulated into 2 PSUM banks
(one per batch) across all supersteps; host extracts the 7x7 block sums.
"""
import os
import numpy as np

LAST_EXEC_NS = []
LAST_TRACE = []

B, C, H, W = 16, 3, 320, 448
HW = H * W
N_ITERS = 5
LAMBDA = 0.01
HUBER_DELTA = 0.1
EPS = 1e-6

BH = 16            # band rows per chunk
CW = 224           # band cols per chunk
N = BH * CW        # 3584 px per chunk
A = N // 128       # 28 mod-128 column blocks
M = N // 16        # 224 wrapped cols
NS = 10            # supersteps
TR = 51            # table rows (16 + 17 + 18)
PW = 250           # table row width in pixels (incl. 1 halo px each side)
RPR = PW - 1       # records per row (249)
TSZ = TR * RPR * 2 # gather-window elements per partition (pair-dup)
YPAD = 17
XOFF = (-1, 199)   # first pixel x of the (left, right) half-plane copies
QROWS = 131        # pair-dup rows per quarter-halo table slab (host pdq)
WROWS = 67         # rows per on-chip sliding window tile (covers 2 supersteps)
WSZ = WROWS * RPR * 2


def skew3(w):
    x, y, z = w[..., 0], w[..., 1], w[..., 2]
    O = np.zeros_like(x)
    return np.stack([np.stack([O, -z, y], -1),
                     np.stack([z, O, -x], -1),
                     np.stack([-y, x, O], -1)], -2)


def se3_exp(xi):
    xi = np.asarray(xi, np.float64)
    v, w = xi[:, :3], xi[:, 3:]
    th2 = np.sum(w * w, -1)[:, None, None]
    th2c = np.maximum(th2, 1e-16)
    th = np.sqrt(th2c)
    small = th2 < 1e-10
    Aa = np.where(small, 1.0 - th2 / 6.0, np.sin(th) / th)
    Bc = np.where(small, 0.5 - th2 / 24.0, (1.0 - np.cos(th)) / th2c)
    Cc = np.where(small, 1.0 / 6.0 - th2 / 120.0, (1.0 - Aa) / th2c)
    K = skew3(w)
    K2 = K @ K
    I = np.eye(3)
    R = I + Aa * K + Bc * K2
    V = I + Bc * K + Cc * K2
    t = np.einsum('bij,bj->bi', V, v)
    T = np.zeros((xi.shape[0], 4, 4))
    T[:, :3, :3] = R
    T[:, :3, 3] = t
    T[:, 3, 3] = 1.0
    return T.astype(np.float32)


def feature_gradient(img):
    p = np.pad(img, ((0, 0), (0, 0), (0, 0), (1, 1)), mode='edge')
    dx = 0.5 * (p[..., 2:] - p[..., :-2])
    p = np.pad(img, ((0, 0), (0, 0), (1, 1), (0, 0)), mode='edge')
    dy = 0.5 * (p[..., 2:, :] - p[..., :-2, :])
    return dx.astype(np.float32), dy.astype(np.float32)


def chunk_of(g, s):
    # xh-epoch schedule: supersteps 0-4 left half, 5-9 right half; group
    # quarter q owns bands 5q..5q+4 so its tables come from one contiguous
    # quarter-halo slab.
    b = g // 4
    q = g % 4
    xh = s // 5
    k = s % 5
    return b, 5 * q + k, xh


def rbase_of(yb):
    # unclipped image-row base of the gather window (padded-row space on host)
    return yb * BH - YPAD


def mod128_k(planes, g, s):
    """planes [2,K,H,W] -> [128, K, A] for chunk (g,s): plane k at pixel
    j=a*128+p lives at [p, k, a]."""
    b, yb, xh = chunk_of(g, s)
    r0, c0 = yb * BH, xh * CW
    blk = planes[b, :, r0:r0 + BH, c0:c0 + CW].reshape(-1, N)    # [K, N]
    return blk.reshape(-1, A, 128).transpose(2, 0, 1)            # [128, K, A]


def host_precompute(pose_twist2, I0_2, I1_2, invD0_2, invD1_2, intr2):
    T0 = se3_exp(pose_twist2)
    fx = intr2[:, 0][:, None, None]; fy = intr2[:, 1][:, None, None]
    cx = intr2[:, 2][:, None, None]; cy = intr2[:, 3][:, None, None]
    uu = np.arange(W, dtype=np.float32)[None, None, :]
    vv = np.arange(H, dtype=np.float32)[None, :, None]
    iD = np.maximum(invD1_2[:, 0], EPS).astype(np.float32)
    z1 = (1.0 / iD).astype(np.float32)
    x1 = ((uu - cx) / fx * z1).astype(np.float32)
    y1 = ((vv - cy) / fy * z1).astype(np.float32)
    R0, t0 = T0[:, :3, :3], T0[:, :3, 3]
    X0 = np.einsum('bij,bhwj->bhwi', R0, np.stack([x1, y1, z1], -1)) + t0[:, None, None, :]
    X0 = X0.astype(np.float32)
    z0 = X0[..., 2]
    z0s = np.where(np.abs(z0) > EPS, z0, EPS).astype(np.float32)
    iz = (1.0 / z0s).astype(np.float32)
    xh_, yh_ = X0[..., 0], X0[..., 1]
    O = np.zeros_like(z0)
    Jp = np.stack([np.stack([fx * iz + O, O, -fx * xh_ * iz * iz], -1),
                   np.stack([O, fy * iz + O, -fy * yh_ * iz * iz], -1)], -2).astype(np.float32)
    I3 = np.broadcast_to(np.eye(3, dtype=np.float32), X0.shape[:3] + (3, 3))
    Jt = np.concatenate([I3, -skew3(X0)], -1).astype(np.float32)
    Jw = np.einsum('bhwij,bhwjk->bhwik', Jp, Jt).astype(np.float32)
    A6 = (-Jw[..., 0, :]).astype(np.float32)          # [2,H,W,6]
    B6 = (-Jw[..., 1, :]).astype(np.float32)
    # T6 = Jt[...,2,:] = (0,0,1, X0y, -X0x, 0); only components 3,4 are stored
    T3 = X0[..., 1].astype(np.float32)
    T4 = (-X0[..., 0]).astype(np.float32)

    dI0x, dI0y = feature_gradient(I0_2)
    dD0x, dD0y = feature_gradient(invD0_2)
    planes12 = np.concatenate([dI0x, dI0y, dD0x, dD0y, I0_2, invD0_2], axis=1).astype(np.float32)

    # pdq: per-partition quarter-halo pair-dup tables [128, 2, QROWS*2*RPR] fp16.
    # Partition 16g+r holds plane r of batch g//4, quarter g%4; row j of the
    # slab is padded-plane row 80q+j (padded rows = image rows + 17 top pad).
    padded = np.concatenate([planes12[..., :1], planes12, planes12[..., -1:]], axis=-1)
    P2 = np.zeros((2, 12, 2, H + 35, 2 * RPR), np.float16)
    for xh in range(2):
        c0 = XOFF[xh] + 1                        # index into padded (x = c - 1)
        half = padded[..., c0:c0 + PW]           # [2,12,H,PW]
        P2[:, :, xh, YPAD:YPAD + H, 0::2] = half[..., :-1]
        P2[:, :, xh, YPAD:YPAD + H, 1::2] = half[..., 1:]
        P2[:, :, xh, :YPAD] = P2[:, :, xh, YPAD:YPAD + 1]
        P2[:, :, xh, YPAD + H:] = P2[:, :, xh, YPAD + H - 1:YPAD + H]
    pdq = np.zeros((128, 2, QROWS * 2 * RPR), np.float16)
    for g in range(8):
        b, q = g // 4, g % 4
        nr = min(QROWS, H + 35 - 80 * q)
        for r in range(12):
            pdq[16 * g + r, :, :nr * 2 * RPR] =                 P2[b, r, :, 80 * q:80 * q + nr].reshape(2, -1)
    inp = {"pdq": np.ascontiguousarray(pdq.reshape(128, 2 * QROWS * 2 * RPR))}

    # mod-128 k-major streams
    ABT = np.stack([A6[..., k] for k in range(6)]
                   + [B6[..., k] for k in range(6)] + [T3, T4], axis=1)  # [2,14,H,W]
    X1 = np.stack([x1, y1, z1], 1)                                      # [2,3,H,W]
    abtm = np.zeros((128, NS, 14 * A * 8), np.float16)
    x1m = np.zeros((128, NS, 3 * A * 8), np.float32)
    i1m = np.zeros((128, NS, 3 * A * 8), np.float16)
    x1w = np.zeros((128, NS, 3 * M), np.float32)
    bw = np.zeros((128, NS, 3), np.float32)
    for s in range(NS):
        for g in range(8):
            b, yb, xh = chunk_of(g, s)
            abtm[:, s, g * 14 * A:(g + 1) * 14 * A] = \
                mod128_k(ABT, g, s).reshape(128, 14 * A).astype(np.float16)
            x1m[:, s, g * 3 * A:(g + 1) * 3 * A] = mod128_k(X1, g, s).reshape(128, 3 * A)
            i1m[:, s, g * 3 * A:(g + 1) * 3 * A] = \
                mod128_k(I1_2[:, :3], g, s).reshape(128, 3 * A).astype(np.float16)
            r0, c0 = yb * BH, xh * CW
            blk = X1[b, :, r0:r0 + BH, c0:c0 + CW].reshape(3, N)
            wr = blk.reshape(3, M, 16).transpose(2, 0, 1)  # [16, 3, M]
            x1w[16 * g:16 * g + 16, s, :] = wr.reshape(16, 3 * M)
            bw[16 * g:16 * g + 16, s, 0] = rbase_of(yb)
            bw[16 * g:16 * g + 16, s, 1] = XOFF[xh]             # xlo
            bw[16 * g:16 * g + 16, s, 2] = XOFF[xh] + PW - 2    # xhi
    inp["abtm"] = np.ascontiguousarray(abtm.reshape(128, NS * 14 * A * 8))
    inp["x1m"] = np.ascontiguousarray(x1m.reshape(128, NS * 3 * A * 8))
    inp["i1m"] = np.ascontiguousarray(i1m.reshape(128, NS * 3 * A * 8))
    inp["x1w"] = np.ascontiguousarray(x1w.reshape(128, NS * 3 * M))
    inp["bw"] = np.ascontiguousarray(bw.reshape(128, NS * 3))
    inp["idn"] = np.eye(128, dtype=np.float16)
    return inp, dict(T0=T0)


def host_iter_params(T2, intr2):
    R = T2[:, :3, :3].astype(np.float32); t = T2[:, :3, 3].astype(np.float32)
    q = np.zeros((2, 16), np.float32)
    q[:, :9] = R.reshape(2, 9)
    q[:, 9:12] = t
    q[:, 12:16] = intr2
    rtm = np.zeros((128, 32), np.float32)
    rtw = np.zeros((128, 16), np.float32)
    for b in range(2):
        rtm[:, b * 16:(b + 1) * 16] = q[b][None, :]
    for g in range(8):
        rtw[16 * g:16 * g + 16, :] = q[g // 4][None, :]
    return {"rtm": np.ascontiguousarray(rtm), "rtw": rtw}


_NC_CACHE = {}


def build_nc(sim_safe=False):
    import concourse.bacc as bacc
    import concourse.bass as bass
    import concourse.tile as tile
    from concourse import mybir

    fp32 = mybir.dt.float32
    fp16 = mybir.dt.float16
    i16 = mybir.dt.int16
    i32 = mybir.dt.int32
    AL = mybir.AluOpType
    ACT = mybir.ActivationFunctionType

    nc = bacc.Bacc("TRN2", target_bir_lowering=False, debug=False, num_devices=8)

    pdq_in = nc.dram_tensor("pdq", [128, 2 * QROWS * 2 * RPR], fp16, kind="ExternalInput")
    abtm_in = nc.dram_tensor("abtm", [128, NS * 14 * A * 8], fp16, kind="ExternalInput")
    x1m_in = nc.dram_tensor("x1m", [128, NS * 3 * A * 8], fp32, kind="ExternalInput")
    i1m_in = nc.dram_tensor("i1m", [128, NS * 3 * A * 8], fp16, kind="ExternalInput")
    x1w_in = nc.dram_tensor("x1w", [128, NS * 3 * M], fp32, kind="ExternalInput")
    bw_in = nc.dram_tensor("bw", [128, NS * 3], fp32, kind="ExternalInput")
    rtm_in = nc.dram_tensor("rtm", [128, 32], fp32, kind="ExternalInput")
    rtw_in = nc.dram_tensor("rtw", [128, 16], fp32, kind="ExternalInput")
    idn_in = nc.dram_tensor("idn", [128, 128], fp16, kind="ExternalInput")
    out_ext = nc.dram_tensor("out", [128, 56], fp32, kind="ExternalOutput")

    TT = nc.vector.tensor_tensor
    TS = lambda out, in0, s1, op: nc.vector.tensor_scalar(out, in0, s1, None, op)
    TS2 = lambda out, in0, s1, s2, op0, op1: nc.vector.tensor_scalar(out, in0, s1, s2, op0, op1)
    STT = nc.vector.scalar_tensor_tensor

    with tile.TileContext(nc) as tc:
        with tc.tile_pool(name="cst", bufs=1) as cpool, \
             tc.tile_pool(name="tblp", bufs=1) as tpool, \
             tc.tile_pool(name="gath", bufs=2) as gp, \
             tc.tile_pool(name="strm", bufs=2) as sp, \
             tc.tile_pool(name="strm1", bufs=1) as sp1, \
             tc.tile_pool(name="xb", bufs=2) as xb, \
             tc.tile_pool(name="scr", bufs=1) as sc, \
             tc.tile_pool(name="ps", bufs=1, space="PSUM") as pp, \
             tc.tile_pool(name="jw", bufs=1, space="PSUM") as jwp:

            rtm = cpool.tile([128, 32], fp32, tag="rtm")
            rtw = cpool.tile([128, 16], fp32, tag="rtw")
            bwc = cpool.tile([128, NS * 3], fp32, tag="bw")
            idn = cpool.tile([128, 128], fp16, tag="idn")
            nc.sync.dma_start(out=rtm[:, :], in_=rtm_in.ap())
            nc.sync.dma_start(out=rtw[:, :], in_=rtw_in.ap())
            nc.sync.dma_start(out=bwc[:, :], in_=bw_in.ap())
            nc.sync.dma_start(out=idn[:, :], in_=idn_in.ap())

            jwt = [jwp.tile([128, 28], fp32, name=f"jw{b}", tag=f"jw{b}") for b in range(2)]

            def rqw(k):   # wrapped per-partition scalar [P,1]
                return rtw[:, k:k + 1]

            def rqm(k, b):  # mod-128 per-batch scalar [P,1]
                return rtm[:, b * 16 + k:b * 16 + k + 1]

            def bwq(s, j):
                return bwc[:, s * 3 + j:s * 3 + j + 1]

            # per-superstep state handed from stage1 (idx+warp) to stage2
            st = [None] * NS

            def stage1(s):
                S = {}
                x1w = sp1.tile([128, 3 * M], fp32, name=f"x1w_{s}", tag="x1w")
                nc.sync.dma_start(out=x1w[:, :], in_=x1w_in.ap()[:, s * 3 * M:(s + 1) * 3 * M])
                x1m = sp1.tile([128, 3 * A * 8], fp32, name=f"x1m_{s}", tag="x1m")
                nc.sync.dma_start(out=x1m[:, :], in_=x1m_in.ap()[:, s * 24 * A:(s + 1) * 24 * A])
                abt = sp.tile([128, 14 * A * 8], fp16, name=f"abt_{s}", tag="abt")
                nc.sync.dma_start(out=abt[:, :], in_=abtm_in.ap()[:, s * 112 * A:(s + 1) * 112 * A])
                i1 = sp.tile([128, 3 * A * 8], fp16, name=f"i1_{s}", tag="i1")
                nc.sync.dma_start(out=i1[:, :], in_=i1m_in.ap()[:, s * 24 * A:(s + 1) * 24 * A])
                S["abt"] = abt; S["i1"] = i1

                xh, k = s // 5, s % 5
                k0 = (k // 2) * 2
                if k % 2 == 0:
                    tblw = tpool.tile([128, WSZ], fp16, name=f"tblw_{s}", tag="tblw")
                    off = xh * QROWS * 2 * RPR + 16 * k0 * 2 * RPR
                    nc.sync.dma_start(out=tblw[:, :],
                                      in_=pdq_in.ap()[:, off:off + WSZ])
                    S["tblw"] = tblw
                else:
                    S["tblw"] = None  # filled from previous superstep below
                woff = 16 * (k - k0) * 2 * RPR

                # ---------- wrapped-16 index pipeline ----------
                def xw(k):
                    sl = x1w[:, :]
                    return bass.AP(sl.tensor, sl.offset + k * M, [list(sl.ap[0]), [1, M]])

                def tw(name):
                    return sc.tile([128, M], fp32, name=f"w_{name}_{s}", tag="w_" + name)

                t1 = tw("t1"); t2 = tw("t2")
                X0z = tw("X0z"); X0x = tw("X0x"); X0y = tw("X0y")
                for dst, r0, r1, r2, tk in ((X0z, 6, 7, 8, 11), (X0x, 0, 1, 2, 9),
                                            (X0y, 3, 4, 5, 10)):
                    nc.vector.tensor_scalar(dst[:, :], xw(2), rqw(r2), rqw(tk), AL.mult, AL.add)
                    STT(dst[:, :], xw(0), rqw(r0), dst[:, :], AL.mult, AL.add)
                    STT(dst[:, :], xw(1), rqw(r1), dst[:, :], AL.mult, AL.add)
                izw = tw("iz")
                TS(t1[:, :], X0z[:, :], EPS, AL.is_gt)
                TS(t2[:, :], X0z[:, :], -EPS, AL.is_lt)
                TT(t2[:, :], t2[:, :], t1[:, :], op=AL.add)
                TT(t1[:, :], X0z[:, :], t2[:, :], op=AL.mult)
                TS2(t2[:, :], t2[:, :], -EPS, EPS, AL.mult, AL.add)
                TT(t1[:, :], t1[:, :], t2[:, :], op=AL.add)
                nc.vector.reciprocal_approx_fast(izw[:, :], t1[:, :])
                u0 = tw("u0"); v0 = tw("v0")
                TT(u0[:, :], X0x[:, :], izw[:, :], op=AL.mult)
                nc.vector.tensor_scalar(u0[:, :], u0[:, :], rqw(12), rqw(14), AL.mult, AL.add)
                TT(v0[:, :], X0y[:, :], izw[:, :], op=AL.mult)
                nc.vector.tensor_scalar(v0[:, :], v0[:, :], rqw(13), rqw(15), AL.mult, AL.add)
                TS2(u0[:, :], u0[:, :], -0.5 * (W - 1), 1.5 * (W - 1), AL.max, AL.min)
                TS2(v0[:, :], v0[:, :], -0.5 * (H - 1), 1.5 * (H - 1), AL.max, AL.min)
                fi = sc.tile([128, M], i32, name=f"w_fi_{s}", tag="w_fi")
                x0f = tw("x0f"); y0f = tw("y0f")
                TS(t1[:, :], u0[:, :], 0.5, AL.subtract)
                nc.vector.tensor_copy(fi[:, :], t1[:, :])
                nc.vector.tensor_copy(x0f[:, :], fi[:, :])
                TS(t1[:, :], v0[:, :], 0.5, AL.subtract)
                nc.vector.tensor_copy(fi[:, :], t1[:, :])
                nc.vector.tensor_copy(y0f[:, :], fi[:, :])
                # kx = clamp(x0f, xlo, xhi) - xlo
                nc.vector.tensor_scalar(t2[:, :], x0f[:, :], bwq(s, 1), bwq(s, 2), AL.max, AL.min)
                nc.vector.tensor_scalar(t2[:, :], t2[:, :], bwq(s, 1), None, AL.subtract)
                yr = tw("yr")
                nc.vector.tensor_scalar(yr[:, :], y0f[:, :], bwq(s, 0), None, AL.subtract)
                kt = tw("kt"); kb = tw("kb")
                TS2(kt[:, :], yr[:, :], 0.0, float(TR - 1), AL.max, AL.min)
                STT(kt[:, :], kt[:, :], float(RPR), t2[:, :], AL.mult, AL.add)
                TS2(kb[:, :], yr[:, :], 1.0, 0.0, AL.add, AL.max)
                TS(kb[:, :], kb[:, :], float(TR - 1), AL.min)
                STT(kb[:, :], kb[:, :], float(RPR), t2[:, :], AL.mult, AL.add)
                kt16 = sc.tile([128, M], i16, name=f"kt16_{s}", tag="kt16")
                kb16 = sc.tile([128, M], i16, name=f"kb16_{s}", tag="kb16")
                nc.vector.tensor_copy(kt16[:, :], kt[:, :])
                nc.vector.tensor_copy(kb16[:, :], kb[:, :])

                gt = gp.tile([128, N * 2], fp16, name=f"gt_{s}", tag="gt")
                gb = gp.tile([128, N * 2], fp16, name=f"gb_{s}", tag="gb")
                tw_ = S["tblw"] if S["tblw"] is not None else st[s - 1]["tblw"]
                S["tblw"] = tw_
                nc.gpsimd.ap_gather(gt[:, :], tw_[:, woff:woff + TSZ], kt16[:, :],
                                    channels=128, num_elems=TR * RPR, d=2, num_idxs=N)
                nc.gpsimd.ap_gather(gb[:, :], tw_[:, woff:woff + TSZ], kb16[:, :],
                                    channels=128, num_elems=TR * RPR, d=2, num_idxs=N)
                S["gt"] = gt; S["gb"] = gb

                # ---------- mod-128 warp pipeline ----------
                def xm(k):
                    sl = x1m[:, :]
                    return bass.AP(sl.tensor, sl.offset + k * A, [list(sl.ap[0]), [3 * A, 8], [1, A]])

                def xmh(k, b):  # batch half: groups 4b..4b+3
                    sl = x1m[:, :]
                    return bass.AP(sl.tensor, sl.offset + b * 12 * A + k * A,
                                   [list(sl.ap[0]), [3 * A, 4], [1, A]])

                def tm(name, dt=fp32):
                    return sc.tile([128, 8 * A], dt, name=f"m_{name}_{s}", tag="m_" + name)

                def half(t, b):
                    return t[:, b * 4 * A:(b + 1) * 4 * A]

                m1 = tm("m1"); m2 = tm("m2")
                X0zm = tm("X0z"); X0xm = tm("X0x"); X0ym = tm("X0y")
                for dst, r0, r1, r2, tk in ((X0zm, 6, 7, 8, 11), (X0xm, 0, 1, 2, 9),
                                            (X0ym, 3, 4, 5, 10)):
                    for b in range(2):
                        hd = half(dst, b)
                        nc.vector.tensor_scalar(hd, xmh(2, b), rqm(r2, b), rqm(tk, b), AL.mult, AL.add)
                        STT(hd, xmh(0, b), rqm(r0, b), hd, AL.mult, AL.add)
                        STT(hd, xmh(1, b), rqm(r1, b), hd, AL.mult, AL.add)
                iz = xb.tile([128, 8 * A], fp32, name=f"m_izk_{s}", tag="m_izk")
                TS(m1[:, :], X0zm[:, :], EPS, AL.is_gt)
                TS(m2[:, :], X0zm[:, :], -EPS, AL.is_lt)
                TT(m2[:, :], m2[:, :], m1[:, :], op=AL.add)
                TT(m1[:, :], X0zm[:, :], m2[:, :], op=AL.mult)
                TS2(m2[:, :], m2[:, :], -EPS, EPS, AL.mult, AL.add)
                TT(m1[:, :], m1[:, :], m2[:, :], op=AL.add)
                nc.vector.reciprocal_approx_fast(iz[:, :], m1[:, :])
                u0m = tm("u0"); v0m = tm("v0")
                TT(u0m[:, :], X0xm[:, :], iz[:, :], op=AL.mult)
                TT(v0m[:, :], X0ym[:, :], iz[:, :], op=AL.mult)
                for b in range(2):
                    nc.vector.tensor_scalar(half(u0m, b), half(u0m, b), rqm(12, b), rqm(14, b), AL.mult, AL.add)
                    nc.vector.tensor_scalar(half(v0m, b), half(v0m, b), rqm(13, b), rqm(15, b), AL.mult, AL.add)
                vm = xb.tile([128, 8 * A], fp16, name=f"m_vm_{s}", tag="m_vm")
                TS(vm[:, :], X0zm[:, :], EPS, AL.is_gt)
                TS(m1[:, :], u0m[:, :], 0.0, AL.is_gt)
                TT(vm[:, :], vm[:, :], m1[:, :], op=AL.mult)
                TS(m1[:, :], u0m[:, :], float(W - 1), AL.is_lt)
                TT(vm[:, :], vm[:, :], m1[:, :], op=AL.mult)
                TS(m1[:, :], v0m[:, :], 0.0, AL.is_gt)
                TT(vm[:, :], vm[:, :], m1[:, :], op=AL.mult)
                TS(m1[:, :], v0m[:, :], float(H - 1), AL.is_lt)
                TT(vm[:, :], vm[:, :], m1[:, :], op=AL.mult)
                one_m = xb.tile([128, 8 * A], fp16, name=f"m_onem_{s}", tag="m_onem")
                TS2(one_m[:, :], vm[:, :], 1.0, -1e-6, AL.subtract, AL.mult)
                S["vm"] = vm; S["one_m"] = one_m; S["iz"] = iz
                TS2(u0m[:, :], u0m[:, :], -0.5 * (W - 1), 1.5 * (W - 1), AL.max, AL.min)
                TS2(v0m[:, :], v0m[:, :], -0.5 * (H - 1), 1.5 * (H - 1), AL.max, AL.min)
                fim = sc.tile([128, 8 * A], i32, name=f"m_fi_{s}", tag="m_fi")
                x0fm = tm("x0f"); y0fm = tm("y0f")
                wx = tm("wx", fp16); wy = tm("wy", fp16)
                TS(m1[:, :], u0m[:, :], 0.5, AL.subtract)
                nc.vector.tensor_copy(fim[:, :], m1[:, :])
                nc.vector.tensor_copy(x0fm[:, :], fim[:, :])
                TT(wx[:, :], u0m[:, :], x0fm[:, :], op=AL.subtract)
                TS(m1[:, :], v0m[:, :], 0.5, AL.subtract)
                nc.vector.tensor_copy(fim[:, :], m1[:, :])
                nc.vector.tensor_copy(y0fm[:, :], fim[:, :])
                TT(wy[:, :], v0m[:, :], y0fm[:, :], op=AL.subtract)
                xf16 = tm("xf16", fp16); yf16 = tm("yf16", fp16)
                nc.vector.tensor_copy(xf16[:, :], x0fm[:, :])
                nc.vector.tensor_copy(yf16[:, :], y0fm[:, :])
                mk = {}
                t16a = tm("t16a", fp16); t16b = tm("t16b", fp16)
                for nm, src16, lo, hi in (("mx0", xf16, -0.5, W - 0.5),
                                          ("mx1", xf16, -1.5, W - 1.5),
                                          ("my0", yf16, -0.5, H - 0.5),
                                          ("my1", yf16, -1.5, H - 1.5)):
                    mt = tm(nm, fp16)
                    TS(t16a[:, :], src16[:, :], float(lo), AL.is_gt)
                    TS(t16b[:, :], src16[:, :], float(hi), AL.is_lt)
                    TT(mt[:, :], t16a[:, :], t16b[:, :], op=AL.mult)
                    mk[nm] = mt
                wxm = tm("wxm", fp16); wym = tm("wym", fp16)
                TS2(wxm[:, :], wx[:, :], 1.0, -1.0, AL.subtract, AL.mult)
                TS2(wym[:, :], wy[:, :], 1.0, -1.0, AL.subtract, AL.mult)
                Wt = []
                for ci, (fx_, fy_, mx_, my_) in enumerate(
                        ((wxm, wym, "mx0", "my0"), (wx, wym, "mx1", "my0"),
                         (wxm, wy, "mx0", "my1"), (wx, wy, "mx1", "my1"))):
                    Wc = xb.tile([128, 8 * A], fp16, name=f"m_W{ci}_{s}", tag=f"m_W{ci}")
                    TT(Wc[:, :], fx_[:, :], fy_[:, :], op=AL.mult)
                    TT(Wc[:, :], Wc[:, :], mk[mx_][:, :], op=AL.mult)
                    TT(Wc[:, :], Wc[:, :], mk[my_][:, :], op=AL.mult)
                    Wt.append(Wc)
                S["W"] = Wt
                return S

            def stage2(s, S):
                gt, gb, Wt = S["gt"], S["gb"], S["W"]
                samp = sc.tile([128, 128 * A], fp16, name=f"samp_{s}", tag="samp")
                ctmp = sc.tile([128, 512], fp16, name=f"ctmp_{s}", tag="ctmp")
                for a4 in range(A // 4):
                    pts = []
                    for ci, (gsrc, e) in enumerate(((gt, 0), (gt, 1), (gb, 0), (gb, 1))):
                        pt = pp.tile([128, 512], fp16, name=f"pt{ci}_{s}_{a4}", tag=f"pt{ci}")
                        pts.append(pt)
                        for aa in range(4):
                            a = a4 * 4 + aa
                            src = bass.AP(gsrc.tensor, gsrc.offset + (a * 128 * 2 + e),
                                          [list(gsrc.ap[0]), [2, 128]])
                            nc.tensor.transpose(pt[:, aa * 128:(aa + 1) * 128], src, idn[:, :])
                    # iteration order (g, q, aa); pt col = aa*128 + 16g + q
                    def pap(pt):
                        return bass.AP(pt.tensor, pt.offset, [list(pt.ap[0]), [16, 8], [1, 12], [128, 4]])
                    def wap(Wc):
                        return bass.AP(Wc.tensor, Wc.offset + a4 * 4,
                                       [list(Wc.ap[0]), [A, 8], [0, 12], [1, 4]])
                    def sap(t, q0=0, nq=12):
                        return bass.AP(t.tensor, t.offset + q0 * A + a4 * 4,
                                       [list(t.ap[0]), [16 * A, 8], [A, nq], [1, 4]])
                    cap = bass.AP(ctmp.tensor, ctmp.offset, [list(ctmp.ap[0]), [64, 8], [4, 12], [1, 4]])
                    TT(sap(samp), pap(pts[0]), wap(Wt[0]), op=AL.mult)
                    for ci in range(1, 4):
                        TT(cap, pap(pts[ci]), wap(Wt[ci]), op=AL.mult)
                        TT(sap(samp), sap(samp), cap, op=AL.add)

                # ---------- residuals, sqrt-huber, scaling ----------
                vm, one_m, iz = S["vm"], S["one_m"], S["iz"]
                abt, i1 = S["abt"], S["i1"]
                jaug = sc.tile([128, 28 * A * 8], fp16, name=f"jaug_{s}", tag="jaug")
                jtmp = sc.tile([128, 6 * A * 8], fp16, name=f"jtmp_{s}", tag="jtmp")
                t32 = sc.tile([128, 8 * A], fp32, name=f"t32_{s}", tag="t32")
                rr = sc.tile([128, 8 * A], fp16, name=f"rr_{s}", tag="rr")
                rz = sc.tile([128, 8 * A], fp32, name=f"rz_{s}", tag="rz")
                sw = [sc.tile([128, 8 * A], fp16, name=f"sw{r}_{s}", tag=f"sw{r}") for r in range(4)]

                def sampq(q):
                    sl = samp[:, :]
                    return bass.AP(sl.tensor, sl.offset + q * A, [list(sl.ap[0]), [16 * A, 8], [1, A]])

                def i1q(c):
                    sl = i1[:, :]
                    return bass.AP(sl.tensor, sl.offset + c * A, [list(sl.ap[0]), [3 * A, 8], [1, A]])

                def jcol(k):   # jaug column (r*7+x) over all (g, a)
                    sl = jaug[:, :]
                    return bass.AP(sl.tensor, sl.offset + k * A, [list(sl.ap[0]), [28 * A, 8], [1, A]])

                for c in range(3):
                    TT(rr[:, :], i1q(c), sampq(8 + c), op=AL.subtract)
                    TT(rr[:, :], rr[:, :], vm[:, :], op=AL.mult)
                    TT(rr[:, :], rr[:, :], one_m[:, :], op=AL.add)
                    nc.scalar.activation(t32[:, :], rr[:, :], ACT.Abs)
                    TS(t32[:, :], t32[:, :], HUBER_DELTA, AL.max)
                    nc.vector.reciprocal_approx_fast(t32[:, :], t32[:, :])
                    nc.scalar.activation(sw[c][:, :], t32[:, :], ACT.Sqrt)
                    TT(jcol(c * 7 + 6), rr[:, :], sw[c][:, :], op=AL.mult)
                TT(rz[:, :], iz[:, :], sampq(11), op=AL.subtract)
                TT(rz[:, :], rz[:, :], vm[:, :], op=AL.mult)
                TT(rz[:, :], rz[:, :], one_m[:, :], op=AL.add)
                nc.scalar.activation(t32[:, :], rz[:, :], ACT.Abs, scale=LAMBDA)
                TS(t32[:, :], t32[:, :], HUBER_DELTA, AL.max)
                nc.vector.reciprocal_approx_fast(t32[:, :], t32[:, :])
                nc.scalar.activation(sw[3][:, :], t32[:, :], ACT.Sqrt, scale=LAMBDA * LAMBDA)
                TT(jcol(3 * 7 + 6), rz[:, :], sw[3][:, :], op=AL.mult)
                # scale sampled gradients by sqrt-weights in place
                for c in range(3):
                    TT(sampq(c), sampq(c), sw[c][:, :], op=AL.mult)
                    TT(sampq(3 + c), sampq(3 + c), sw[c][:, :], op=AL.mult)
                TT(sampq(6), sampq(6), sw[3][:, :], op=AL.mult)
                TT(sampq(7), sampq(7), sw[3][:, :], op=AL.mult)

                # ---------- jaug J-part ----------
                def abtb(k0):   # abt block: 6 planes starting at k0, x-major
                    sl = abt[:, :]
                    return bass.AP(sl.tensor, sl.offset + k0 * A,
                                   [list(sl.ap[0]), [14 * A, 8], [A, 6], [1, A]])

                def abt1(k):
                    sl = abt[:, :]
                    return bass.AP(sl.tensor, sl.offset + k * A, [list(sl.ap[0]), [14 * A, 8], [1, A]])

                def jblk(r):   # jaug 6-col J block of residual row r
                    sl = jaug[:, :]
                    return bass.AP(sl.tensor, sl.offset + (r * 7) * A,
                                   [list(sl.ap[0]), [28 * A, 8], [A, 6], [1, A]])

                def sampb(q):  # samp slot q broadcast over 6 x's
                    sl = samp[:, :]
                    return bass.AP(sl.tensor, sl.offset + q * A,
                                   [list(sl.ap[0]), [16 * A, 8], [0, 6], [1, A]])

                jtap = bass.AP(jtmp.tensor, jtmp.offset, [list(jtmp.ap[0]), [6 * A, 8], [A, 6], [1, A]])
                for r, (qa, qb) in enumerate(((0, 3), (1, 4), (2, 5), (6, 7))):
                    TT(jblk(r), abtb(0), sampb(qa), op=AL.mult)
                    TT(jtap, abtb(6), sampb(qb), op=AL.mult)
                    TT(jblk(r), jblk(r), jtap, op=AL.add)
                # depth row T6 additions: x=2 (+sw), x=3 (+sw*T3), x=4 (+sw*T4)
                TT(jcol(21 + 2), jcol(21 + 2), sw[3][:, :], op=AL.add)
                t16 = sc.tile([128, 8 * A], fp16, name=f"jt16_{s}", tag="jt16")
                TT(t16[:, :], abt1(12), sw[3][:, :], op=AL.mult)
                TT(jcol(21 + 3), jcol(21 + 3), t16[:, :], op=AL.add)
                TT(t16[:, :], abt1(13), sw[3][:, :], op=AL.mult)
                TT(jcol(21 + 4), jcol(21 + 4), t16[:, :], op=AL.add)

                # ---------- PE JtWJ/Rhs accumulation ----------
                for g in range(8):
                    b = g // 4
                    for a in range(A):
                        mm = bass.AP(jaug.tensor, jaug.offset + g * 28 * A + a,
                                     [list(jaug.ap[0]), [A, 28]])
                        start = (s == 0 and g % 4 == 0 and a == 0)
                        stop = (s == NS - 1 and g % 4 == 3 and a == A - 1)
                        nc.tensor.matmul(jwt[b][:28, :28], mm, mm, start=start, stop=stop)

            for s in range(NS):
                st[s] = stage1(s)
                if s >= 1:
                    stage2(s - 1, st[s - 1])
                    st[s - 1] = None
            stage2(NS - 1, st[NS - 1])

            osb = cpool.tile([128, 56], fp32, tag="osb")
            for b in range(2):
                nc.vector.tensor_copy(osb[:28, b * 28:(b + 1) * 28], jwt[b][:28, :28])
            oap = out_ext.ap()
            dst = bass.AP(oap.tensor, oap.offset, [[oap.ap[0][0], 28], [1, 56]])
            nc.sync.dma_start(out=dst, in_=osb[:28, :])

    nc.finalize()
    return nc


def assemble(out_rows):
    """out_rows: [128, 56] -> (JtWJ [2,6,6], Rhs [2,6])."""
    JtWJ = np.zeros((2, 6, 6), np.float32)
    Rhs = np.zeros((2, 6), np.float32)
    for b in range(2):
        M28 = out_rows[:28, b * 28:(b + 1) * 28]
        M7 = sum(M28[7 * r:7 * r + 7, 7 * r:7 * r + 7] for r in range(4))
        JtWJ[b] = M7[:6, :6]
        Rhs[b] = M7[:6, 6]
    return JtWJ, Rhs


def solve_update(T2, JtWJ, Rhs):
    tr = np.trace(JtWJ, axis1=-2, axis2=-1)
    Hm = JtWJ + (tr * 1e-6)[:, None, None] * np.eye(6, dtype=np.float32)
    xi = np.linalg.solve(Hm.astype(np.float64), Rhs.astype(np.float64)[..., None])[..., 0]
    return (T2 @ se3_exp(-xi)).astype(np.float32)


def kernel(pose_twist, I0, I1, invD0, invD1, intrinsics):
    from concourse.bass_utils import run_bass_kernel_spmd

    nc = _NC_CACHE.get("nc")
    if nc is None:
        nc = build_nc()
        _NC_CACHE["nc"] = nc

    pose_twist = np.asarray(pose_twist, np.float32)
    I0 = np.asarray(I0, np.float32); I1 = np.asarray(I1, np.float32)
    invD0 = np.asarray(invD0, np.float32); invD1 = np.asarray(invD1, np.float32)
    intrinsics = np.asarray(intrinsics, np.float32)

    core_inputs = []
    T_cur = []
    for core in range(8):
        sl = slice(2 * core, 2 * core + 2)
        inp, stt = host_precompute(pose_twist[sl], I0[sl], I1[sl], invD0[sl],
                                   invD1[sl], intrinsics[sl])
        core_inputs.append(inp)
        T_cur.append(stt["T0"])

    profile = bool(os.environ.get("BASS_ALIGN_PROFILE"))
    for it in range(N_ITERS):
        in_maps = []
        for core in range(8):
            m = dict(core_inputs[core])
            m.update(host_iter_params(T_cur[core], intrinsics[2 * core:2 * core + 2]))
            in_maps.append(m)
        kw = {}
        if profile and it == 0:
            pdir = f"/tmp/align_prof/it{it}"
            os.makedirs(pdir, exist_ok=True)
            kw = dict(trace=True, tmpdir=pdir)
        res = run_bass_kernel_spmd(nc, in_maps, list(range(8)), **kw)
        if profile and it == 0:
            LAST_EXEC_NS.append(res.exec_time_ns)
            LAST_TRACE.append(res.instructions_and_trace)
        for core in range(8):
            JtWJ, Rhs = assemble(res.results[core]["out"])
            T_cur[core] = solve_update(T_cur[core], JtWJ, Rhs)

    return np.concatenate(T_cur, axis=0).astype(np.float32)
